# revision 2
# baseline (speedup 1.0000x reference)
"""APPNP (k=1) GNN kernel for 8 TRN2 NeuronCores.

Math (per reference.py):
    x    = relu(features @ W.T + b)                       [N, 16]
    h    = x * rsqrt(clip(out_deg, 1))[:, None]
    agg  = segment_sum(h[src], dst)                       [N, 16]
    out  = 0.8 * agg * rsqrt(clip(in_deg, 1))[:, None] + 0.2 * x

Sharding: nodes split into 8 contiguous blocks of NB=12500. Core c owns
block c: computes x/h for its block, AllGathers h (6.4 MB), then
aggregates the edges whose dst lands in its block.

Per-core aggregation pipeline:
  - edges grouped by (src_chunk g, dst segment s), dst-sorted inside a block
  - gpsimd.ap_gather pulls h[src] from the SBUF-resident table
    (partition rows 16g+c hold channel c of chunk g)
  - DVE tensor_tensor_scan computes a prefix sum over each block's
    message stream (channels ride the partitions)
  - a second ap_gather samples the prefix at per-node run-end positions;
    adjacent differences give per-(group, node) partial sums
  - one PE matmul with a 0/1 selector sums the 8 groups' partials
  - final mix on DVE in a folded [128, JF] layout
"""

import sys
sys.path.insert(0, '/opt/trn_rl_repo')

import numpy as np
from concourse import bass, bacc, tile, mybir, bass_utils

F32 = mybir.dt.float32
F32R = mybir.dt.float32r
BF16 = mybir.dt.bfloat16
I16 = mybir.dt.int16

# ---------------------------------------------------------------- params
class P:
    CORES = 8
    N = 100000
    NB = N // CORES            # 12500 nodes per core block
    C = 16                     # output channels
    F = 512                    # input features
    G = 8                      # groups = src chunks
    NR = 640                   # nodes per boundary segment (mult of 16)
    S = -1                     # segments  = ceil(NB / NR) (set below)
    NODES_PAD = -1             # S * NR
    NP = 12544                 # padded node count for mix: 128*98 = 8*1568
    JF = NP // 8               # 1568 mix free dim
    NTILE = 500                # node cols per linear-phase tile
    KCH = 4                    # 512 / 128 contraction chunks
    SEG = -1                   # edge slots per (group, segment) block (data-dep)
    SS = 4                     # segments per superseg
    NSS = -1                   # supersegs
    SSEG = -1                  # SS * SEG

P.S = (P.NB + P.NR - 1) // P.NR
P.NODES_PAD = P.S * P.NR
assert P.S % P.SS == 0
P.NSS = P.S // P.SS
assert P.NP % 8 == 0 and 128 * (P.NP // 128) == P.NP and P.NP >= P.NB
assert P.NB % P.NTILE == 0


# ---------------------------------------------------------------- host prep
def host_prep(features, W, b, src_idx, dst_idx):
    """Slice/sort/pad the inputs into per-core device arrays."""
    CORES, NB, C, F, G, NR, S = P.CORES, P.NB, P.C, P.F, P.G, P.NR, P.S
    src_idx = np.asarray(src_idx)
    dst_idx = np.asarray(dst_idx)

    dst_core = dst_idx // NB
    src_core = src_idx // NB

    per_core = []
    blocks_all = []   # per core: dict (g, s) -> (sorted local srcs, local dsts)
    for c in range(CORES):
        m = dst_core == c
        e_src = src_idx[m]
        e_dstl = dst_idx[m] - c * NB
        g = e_src // NB
        s = e_dstl // NR
        order = np.lexsort((e_dstl, s, g))
        e_src, e_dstl, g, s = e_src[order], e_dstl[order], g[order], s[order]
        key = g.astype(np.int64) * S + s
        # block boundaries in the sorted stream
        blocks = {}
        uniq, starts = np.unique(key, return_index=True)
        starts = list(starts) + [len(key)]
        for i, k in enumerate(uniq):
            sl = slice(starts[i], starts[i + 1])
            blocks[(int(k) // S, int(k) % S)] = (e_src[sl] % NB, e_dstl[sl])
        blocks_all.append(blocks)
        per_core.append((e_src, e_dstl))

    max_block = 0
    for blocks in blocks_all:
        for (src_l, _d) in blocks.values():
            max_block = max(max_block, len(src_l))
    SEG = ((max_block + 2 + 15) // 16) * 16
    P.SEG = SEG
    # packed superseg stream length: max over (core, group, superseg)
    max_pack = 0
    for blocks in blocks_all:
        for gg in range(P.G):
            for sup in range(P.NSS):
                tot = sum(len(blocks.get((gg, sup * P.SS + sl),
                                         ((), ()))[0]) + 1
                          for sl in range(P.SS))
                max_pack = max(max_pack, tot)
    P.SSEG = ((max_pack + 1 + 31) // 32) * 32
    assert P.SSEG < 32767 - SEG

    in_maps = []
    for c in range(CORES):
        blocks = blocks_all[c]
        SSn, NSS, SSEG = P.SS, P.NSS, P.SSEG
        eidx = np.full((NSS, 128, SSEG // 16), NB, dtype=np.int16)  # sentinel -> zero row
        bidx = np.zeros((NSS, 128, (SSn * NR) // 16), dtype=np.int16)
        for gg in range(G):
            row0 = 16 * gg
            for sup in range(NSS):
                run = 0   # packed position within this (group, superseg) stream
                for sl in range(SSn):
                    ss = sup * SSn + sl
                    src_l, dstl = blocks.get((gg, ss), (np.zeros(0, np.int64),
                                                        np.zeros(0, np.int64)))
                    e = len(src_l)
                    # packed stream: one sentinel at `run`, edges at run+1..run+e;
                    # the scan runs continuously, sentinels add 0
                    pos = run + np.arange(1, e + 1)
                    assert pos[-1] < SSEG if e else run < SSEG
                    eidx[sup, row0 + (pos % 16), pos // 16] = src_l.astype(np.int16)
                    # B[j] = P[run + #edges with dst_local <= node]; diffs of
                    # consecutive nodes telescope across block boundaries
                    node_lo = ss * NR
                    ends = run + np.searchsorted(dstl, node_lo + np.arange(NR),
                                                 side='right')
                    j = sl * NR + np.arange(NR)
                    bidx[sup, row0 + (j % 16), j // 16] = ends.astype(np.int16)
                    run += e + 1

        # degree pointer arrays, folded [128, 98] with n = p*98 + j
        def fold_ptrs(sorted_vals):
            lo = np.searchsorted(sorted_vals, np.arange(NB)).astype(np.float32)
            hi = np.searchsorted(sorted_vals, np.arange(NB), side='right').astype(np.float32)
            lo_p = np.zeros(128 * 98, np.float32)
            hi_p = np.zeros(128 * 98, np.float32)
            lo_p[:NB] = lo
            hi_p[:NB] = hi
            return lo_p.reshape(128, 98), hi_p.reshape(128, 98)

        cin_lo, cin_hi = fold_ptrs(np.sort(per_core[c][1]))
        own_src = src_idx[src_core == c] % NB
        cout_lo, cout_hi = fold_ptrs(np.sort(own_src))

        selmat = np.zeros((128, C), np.float32)
        for gg in range(G):
            for cc in range(C):
                selmat[16 * gg + cc, cc] = 1.0

        import ml_dtypes
        featT = np.ascontiguousarray(features[c * NB:(c + 1) * NB].T)  # [512, NB]
        ftt = featT.reshape(P.KCH, 128, P.NB // P.NTILE, P.NTILE)
        ftt = np.ascontiguousarray(ftt.transpose(0, 2, 1, 3))  # [K,T,128,NTILE]
        in_maps.append({
            "featT": ftt.astype(ml_dtypes.bfloat16),
            "wt": np.ascontiguousarray(W.T).astype(ml_dtypes.bfloat16),
            "bvec": np.asarray(b, np.float32).reshape(C, 1),
            "selmat": selmat,
            "eidx": eidx,
            "bidx": bidx,
            "cin_lo": cin_lo, "cin_hi": cin_hi,
            "cout_lo": cout_lo, "cout_hi": cout_hi,
        })
    return in_maps


# ---------------------------------------------------------------- device build
def build_program():
    CORES, NB, C, F, G, NR, S = P.CORES, P.NB, P.C, P.F, P.G, P.NR, P.S
    SEG, NP, JF, NTILE, KCH = P.SEG, P.NP, P.JF, P.NTILE, P.KCH
    NODES_PAD = P.NODES_PAD
    NTILES = NB // NTILE

    nc = bacc.Bacc("TRN2", target_bir_lowering=False, debug=False,
                   num_devices=CORES)

    featT = nc.dram_tensor("featT", [KCH, NTILES, 128, NTILE], BF16, kind="ExternalInput")
    wt = nc.dram_tensor("wt", [F, C], BF16, kind="ExternalInput")
    bvec = nc.dram_tensor("bvec", [C, 1], F32, kind="ExternalInput")
    selmat = nc.dram_tensor("selmat", [128, C], F32, kind="ExternalInput")
    eidx = nc.dram_tensor("eidx", [P.NSS, 128, P.SSEG // 16], I16, kind="ExternalInput")
    bidx = nc.dram_tensor("bidx", [P.NSS, 128, (P.SS * NR) // 16], I16, kind="ExternalInput")
    cin_lo = nc.dram_tensor("cin_lo", [128, 98], F32, kind="ExternalInput")
    cin_hi = nc.dram_tensor("cin_hi", [128, 98], F32, kind="ExternalInput")
    cout_lo = nc.dram_tensor("cout_lo", [128, 98], F32, kind="ExternalInput")
    cout_hi = nc.dram_tensor("cout_hi", [128, 98], F32, kind="ExternalInput")

    y = nc.dram_tensor("y", [128, JF], F32, kind="ExternalOutput")

    cc_in = nc.dram_tensor("cc_in", [C, NB], BF16)
    cc_out = nc.dram_tensor("cc_out", [128, NB], BF16, addr_space="Shared")
    x_dram = nc.dram_tensor("x_dram", [C, NP], F32)
    agg_dram = nc.dram_tensor("agg_dram", [C, NP], F32)
    nsrc_dram = nc.dram_tensor("nsrc_dram", [128, 98], F32)
    ndst_dram = nc.dram_tensor("ndst_dram", [128, 98], F32)

    AF = mybir.ActivationFunctionType
    OP = mybir.AluOpType

    with tile.TileContext(nc) as tc:
        # ---- norms: norm = scale_sqrt(1 / clip(hi - lo, 1)) -> DRAM
        with tc.tile_pool(name="norm", bufs=2) as npool:
            for (lo_t, hi_t, out_t, scale) in (
                (cout_lo, cout_hi, nsrc_dram, 1.0),
                (cin_lo, cin_hi, ndst_dram, 0.64),
            ):
                lo = npool.tile([128, 98], F32, tag="lo")
                hi = npool.tile([128, 98], F32, tag="hi")
                nc.sync.dma_start(out=lo[:, :], in_=lo_t[:, :])
                nc.sync.dma_start(out=hi[:, :], in_=hi_t[:, :])
                dg = npool.tile([128, 98], F32, tag="dg")
                nc.vector.tensor_sub(dg[:, :], hi[:, :], lo[:, :])
                nc.vector.tensor_scalar_max(dg[:, :], dg[:, :], 1.0)
                rc = npool.tile([128, 98], F32, tag="rc")
                nc.vector.reciprocal(rc[:, :], dg[:, :])
                nm = npool.tile([128, 98], F32, tag="nm")
                # sqrt(scale / deg) = sqrt(scale) * rsqrt(deg)
                nc.scalar.activation(nm[:, :], rc[:, :], AF.Sqrt, scale=scale)
                nc.sync.dma_start(out=out_t[:, :], in_=nm[:, :])

        nsrc_flat = nsrc_dram.ap().rearrange("a b -> (a b)")

        # ---- linear phase: xT = relu(W @ featT + b); h = xT * nsrc
        with (
            tc.tile_pool(name="wpool", bufs=1) as wpool,
            tc.tile_pool(name="fpool", bufs=3) as fpool,
            tc.tile_pool(name="xpsum", bufs=8, space="PSUM") as xpsum,
            tc.tile_pool(name="spool", bufs=4) as spool,
        ):
            nsall = wpool.tile([C, NB], F32, tag="nsall")
            nc.sync.dma_start(
                out=nsall[:, :],
                in_=nsrc_flat[0:NB][None, :].broadcast_to((C, NB)))
            wts = []
            for k in range(KCH):
                wk = wpool.tile([128, C], BF16, tag=f"w{k}")
                nc.sync.dma_start(out=wk[:, :], in_=wt[k * 128:(k + 1) * 128, :])
                wts.append(wk)
            bt = wpool.tile([C, 1], F32, tag="bt")
            nc.sync.dma_start(out=bt[:, :], in_=bvec[:, :])

            SUPER = 8
            for sup0 in range(0, NTILES, SUPER):
                tiles = range(sup0, min(sup0 + SUPER, NTILES))
                psums = {}
                for t in tiles:
                    psums[t] = xpsum.tile([C, NTILE], F32, tag="xp",
                                          name=f"xp{t}")
                for k in range(KCH):
                    for t in tiles:
                        ft = fpool.tile([128, NTILE], BF16, tag="ft")
                        nc.sync.dma_start(
                            out=ft[:, :],
                            in_=featT[k, t, :, :])
                        nc.tensor.matmul(
                            psums[t][:, :],
                            wts[k][:, :],
                            ft[:, :],
                            start=(k == 0), stop=(k == KCH - 1))
                for t in tiles:
                    cols = slice(t * NTILE, (t + 1) * NTILE)
                    xs = spool.tile([C, NTILE], F32, tag="xs")
                    nc.scalar.activation(xs[:, :], psums[t][:, :], AF.Relu,
                                         bias=bt[:, 0:1])
                    nc.sync.dma_start(out=x_dram[:, cols], in_=xs[:, :])
                    hs = spool.tile([C, NTILE], BF16, tag="hs")
                    nc.vector.tensor_mul(hs[:, :], xs[:, :],
                                         nsall[:, cols])
                    nc.sync.dma_start(out=cc_in[:, cols], in_=hs[:, :])
            # pad x_dram tail
            zpad = spool.tile([C, NP - NB], F32, tag="zp")
            nc.vector.memset(zpad[:, :], 0.0)
            nc.sync.dma_start(out=x_dram[:, NB:NP], in_=zpad[:, :])

        # ---- allgather h
        nc.gpsimd.collective_compute(
            "AllGather", OP.bypass,
            replica_groups=[list(range(CORES))],
            ins=[cc_in.ap().opt()],
            outs=[cc_out.ap().opt()],
        )

        # ---- gather/scan/boundary phase
        with (
            tc.tile_pool(name="tpool", bufs=1) as tpool,
            tc.tile_pool(name="ppool", bufs=1) as ppool,
            tc.tile_pool(name="gpool", bufs=1) as gpool,
            tc.tile_pool(name="prpool", bufs=1) as prpool,
            tc.tile_pool(name="ipool", bufs=3) as ipool,
            tc.tile_pool(name="bpool", bufs=1) as bpool,
        ):
            SSEG, NBR = P.SSEG, P.SS * NR
            table = tpool.tile([128, NB + 1], F32, tag="table")
            CHK = 3200
            for q0 in range(0, NB, CHK):
                q1 = min(q0 + CHK, NB)
                tb = ipool.tile([128, CHK], BF16, tag="tb", name=f"tb{q0}",
                                bufs=1)
                nc.sync.dma_start(out=tb[:, 0:q1 - q0], in_=cc_out[:, q0:q1])
                nc.vector.tensor_copy(table[:, q0:q1], tb[:, 0:q1 - q0])
            nc.vector.memset(table[:, NB:NB + 1], 0.0)
            partials = ppool.tile([128, NODES_PAD], F32, tag="partials")

            for ss in range(P.NSS):
                H = (SSEG // 32) * 16
                eta = ipool.tile([128, H // 16], I16, tag="eta")
                etb = ipool.tile([128, (SSEG - H) // 16], I16, tag="etb")
                nc.sync.dma_start(out=eta[:, :], in_=eidx[ss, :, 0:H // 16])
                nc.sync.dma_start(out=etb[:, :], in_=eidx[ss, :, H // 16:SSEG // 16])
                gat = gpool.tile([128, SSEG], F32, tag="gat")
                pr = prpool.tile([128, SSEG], F32, tag="pr")
                nc.gpsimd.ap_gather(gat[:, 0:H], table[:, :], eta[:, :],
                                    channels=128, num_elems=NB + 1, d=1,
                                    num_idxs=H)
                nc.vector.tensor_tensor_scan(pr[:, 0:H], gat[:, 0:H],
                                             gat[:, 0:H], 0.0,
                                             OP.add, OP.bypass)
                nc.gpsimd.ap_gather(gat[:, H:SSEG], table[:, :], etb[:, :],
                                    channels=128, num_elems=NB + 1, d=1,
                                    num_idxs=SSEG - H)
                nc.vector.tensor_tensor_scan(pr[:, H:SSEG], gat[:, H:SSEG],
                                             gat[:, H:SSEG], 0.0,
                                             OP.add, OP.bypass)
                # chain: add first half's total to the second half's prefix
                nc.vector.tensor_scalar_add(pr[:, H:SSEG], pr[:, H:SSEG],
                                            pr[:, H - 1:H])
                btl = ipool.tile([128, NBR // 16], I16, tag="btl")
                nc.sync.dma_start(out=btl[:, :], in_=bidx[ss, :, :])
                bs = bpool.tile([128, NBR + 1], F32, tag="bs")
                nc.vector.memset(bs[:, 0:1], 0.0)
                nc.gpsimd.ap_gather(bs[:, 1:NBR + 1], pr[:, :], btl[:, :],
                                    channels=128, num_elems=SSEG, d=1,
                                    num_idxs=NBR)
                nc.vector.tensor_sub(partials[:, ss * NBR:(ss + 1) * NBR],
                                     bs[:, 1:NBR + 1], bs[:, 0:NBR])

            # ---- group reduce: agg[c, n] = sum_g partials[16g+c, n]
            with (
                tc.tile_pool(name="apsum", bufs=2, space="PSUM") as apsum,
                tc.tile_pool(name="rpool", bufs=2) as rpool,
            ):
                sel = rpool.tile([128, C], F32, tag="sel")
                nc.sync.dma_start(out=sel[:, :], in_=selmat[:, :])
                done = 0
                while done < NP:
                    n = min(512, NP - done)
                    ap = apsum.tile([C, 512], F32, tag="ap")
                    nc.tensor.matmul(ap[:, 0:n],
                                     sel[:, :],
                                     partials[:, done:done + n],
                                     start=True, stop=True)
                    ast = rpool.tile([C, 512], F32, tag="ast")
                    nc.scalar.activation(ast[:, 0:n], ap[:, 0:n], AF.Copy)
                    nc.sync.dma_start(out=agg_dram[:, done:done + n],
                                      in_=ast[:, 0:n])
                    done += n

        # ---- final mix in folded [128, JF] layout
        with tc.tile_pool(name="mpool", bufs=1) as mpool:
            tagg = mpool.tile([128, JF], F32, tag="tagg")
            tx = mpool.tile([128, JF], F32, tag="tx")
            tnd = mpool.tile([128, JF], F32, tag="tnd")
            agg_flat = agg_dram.ap().rearrange("a b -> (a b)")
            x_flat = x_dram.ap().rearrange("a b -> (a b)")
            nd_flat = ndst_dram.ap().rearrange("a b -> (a b)")
            nc.sync.dma_start(
                out=tagg[:, :],
                in_=agg_flat.rearrange("(p j) -> p j", p=128))
            nc.sync.dma_start(
                out=tx[:, :],
                in_=x_flat.rearrange("(p j) -> p j", p=128))
            nc.sync.dma_start(
                out=tnd[:, :],
                in_=nd_flat[0:NP].rearrange("(s j) -> s j", s=8)[None, :, :]
                    .broadcast_to((C, 8, JF)))
            nc.vector.tensor_mul(tagg[:, :], tagg[:, :], tnd[:, :])
            tout = mpool.tile([128, JF], F32, tag="tout")
            nc.vector.scalar_tensor_tensor(tout[:, :], tx[:, :], 0.2,
                                           tagg[:, :], OP.mult, OP.add)
            nc.sync.dma_start(out=y[:, :], in_=tout[:, :])

    nc.compile()
    return nc


# ---------------------------------------------------------------- unshard
def unshard(results):
    out = np.empty((P.N, P.C), np.float32)
    for c in range(P.CORES):
        yc = results[c]["y"]                      # [128, JF]
        flat = np.ascontiguousarray(yc).reshape(-1)      # c-major [C, NP]
        xc = flat.reshape(P.C, P.NP)[:, :P.NB]
        out[c * P.NB:(c + 1) * P.NB, :] = xc.T
    return out


def run(features, W, b, src_idx, dst_idx, trace=False):
    in_maps = host_prep(features, W, b, src_idx, dst_idx)
    nc = build_program()
    res = bass_utils.run_bass_kernel_spmd(
        nc, in_maps, core_ids=list(range(P.CORES)), trace=trace)
    return unshard(res.results), res


# ---------------------------------------------------------------- entry point
_cache = {}

def kernel(features, W, b, src_idx, dst_idx):
    """Full-input APPNP kernel: shards internally across 8 NeuronCores."""
    import numpy as np
    features = np.asarray(features, dtype=np.float32)
    W = np.asarray(W, dtype=np.float32)
    b = np.asarray(b, dtype=np.float32)
    src_idx = np.asarray(src_idx, dtype=np.int32)
    dst_idx = np.asarray(dst_idx, dtype=np.int32)
    in_maps = host_prep(features, W, b, src_idx, dst_idx)
    key = ("prog", P.SEG, P.SSEG)
    if key not in _cache:
        _cache[key] = build_program()
    nc = _cache[key]
    res = bass_utils.run_bass_kernel_spmd(
        nc, in_maps, core_ids=list(range(P.CORES)), trace=False)
    return unshard(res.results)

